# revision 1
# baseline (speedup 1.0000x reference)
"""Trainium2 Bass kernel for nn_CosSimConv2D.

Math (per sample b):
  s    = im2col3x3(x) @ w_hat           where w_hat = w / (||w||_col + qv)
  out  = sign(s) * exp(a_u/2 * (ln(s^2) - ln(box)))
  box  = 3x3 box-filter of per-pixel sum(x^2)  (= ||im2col row||^2)
  a    = softmax(p)
(The eps=1e-12 terms of the reference are dropped; they are ~1e-7-relative.)

GEMM precision: x and w_hat are each split hi+lo in bf16; three product
terms (xh@wh + xh@wl + xl@wh) recover ~fp32-grade dot products.
Data-parallel over batch: core b computes sample b.

Single fused streaming pass over 8 slabs of 16 image rows:
  iteration s: prefetch DMA(s+1); square+reduce slab s into the s2 image;
  hi/lo-pack + PE-transpose slab s into the persistent padded images;
  box-filter/ln/partition-broadcast for the 4 output tiles enabled by
  slab s; then 15 GEMM matmuls + epilogue per tile.
Epilogue engine split: DVE does s^2 and bitwise sign, scalar does only
Ln and Exp (act table set 6 preloaded once - contains both), GpSimd does
the lbc subtract, the sign multiply, and the lbc partition broadcast.

Layouts on device (per core):
  alloc1 (128p, 130*130) bf16 : partitions 0-63 = x_hi^T padded image,
                                partitions 64-127 = x_lo^T padded image
  alloc2 (128p, 130*130) bf16 : partitions 0-63 = x_hi^T,
                                partitions 64-127 = x_hi^T shifted +1 col
  out tiles: (128 units, 512 pixels) in PSUM -> epilogue -> DRAM (128u, 16384pix)
Host transposes the per-core result back to (H, W, UNITS).
"""

import sys

sys.path.insert(0, "/opt/trn_rl_repo")

import numpy as np
import ml_dtypes

import concourse.bass as bass
import concourse.mybir as mybir
import concourse.tile as tile
from concourse import bacc
from concourse.bass_utils import run_bass_kernel_spmd
from concourse.masks import make_identity

BF16 = mybir.dt.bfloat16
F32 = mybir.dt.float32
U32 = mybir.dt.uint32
AF = mybir.ActivationFunctionType

B, H, W, C, UNITS = 8, 128, 128, 64, 128
HW = H * W  # 16384
HP, WP = H + 2, W + 2  # 130x130 padded image
NTAP = 9
SLAB = 16  # image rows per streaming slab
NSLAB = H // SLAB
TILE_ROWS = 4  # image rows per output tile -> N = 512
NT = H // TILE_ROWS  # 32 output tiles
NPIX = TILE_ROWS * W  # 512

_CACHE = {}


def _build():
    nc = bacc.Bacc("TRN2", target_bir_lowering=False, debug=False)

    x_d = nc.dram_tensor("x", [HW, 2, C], BF16, kind="ExternalInput")
    wt13_d = nc.dram_tensor("wt13", [NTAP, 128, UNITS], BF16, kind="ExternalInput")
    wt2p_d = nc.dram_tensor("wt2p", [3, 128, UNITS], BF16, kind="ExternalInput")
    ws_last_d = nc.dram_tensor("ws_last", [3, 64, UNITS], BF16, kind="ExternalInput")
    a2_d = nc.dram_tensor("a2", [128, 1], F32, kind="ExternalInput")
    band_d = nc.dram_tensor("band", [128, 128], BF16, kind="ExternalInput")
    out_d = nc.dram_tensor("out", [128, HW], F32, kind="ExternalOutput")

    with tile.TileContext(nc) as tc:
        with (
            tc.tile_pool(name="const", bufs=1) as constp,
            tc.tile_pool(name="big", bufs=1) as bigp,
            tc.tile_pool(name="slab", bufs=3) as slabp,
            tc.tile_pool(name="box", bufs=2) as boxp_pool,
            tc.tile_pool(name="epi", bufs=3) as epip,
            tc.tile_pool(name="ptr", bufs=2, space="PSUM") as ptrp,
            tc.tile_pool(name="pmm", bufs=3, space="PSUM") as pmmp,
            tc.tile_pool(name="pmisc", bufs=1, space="PSUM") as pmiscp,
            tc.tile_pool(name="pwarm", bufs=1, space="PSUM") as pwarmp,
        ):
            # ---- input prefetch first: the sync queue issues DMAs
            # serially (~0.7us each), so the latency-critical input slabs
            # go ahead of the constants.
            xv = x_d.ap().rearrange("(h w) t c -> h w t c", w=W)
            packed_tiles = {}

            def issue_dma(s):
                if s >= NSLAB:
                    return
                t = slabp.tile([128, SLAB, 2, C], BF16, tag="packed")
                nc.sync.dma_start(
                    out=t,
                    in_=xv[s * SLAB : (s + 1) * SLAB].rearrange(
                        "h w t c -> w h t c"
                    ),
                )
                packed_tiles[s] = t

            issue_dma(0)
            issue_dma(1)

            # ---- constants ----
            wt13 = constp.tile([128, NTAP, UNITS], BF16, tag="wt13")
            nc.sync.dma_start(out=wt13, in_=wt13_d.ap().rearrange("t k u -> k t u"))
            wt2p = constp.tile([128, 3, UNITS], BF16, tag="wt2p")
            nc.sync.dma_start(out=wt2p, in_=wt2p_d.ap().rearrange("t k u -> k t u"))
            ws_last = constp.tile([64, 3, UNITS], BF16, tag="wsl")
            nc.sync.dma_start(out=ws_last, in_=ws_last_d.ap().rearrange("t k u -> k t u"))
            a2 = constp.tile([128, 1], F32, tag="a2")
            nc.sync.dma_start(out=a2, in_=a2_d[:, :])
            band = constp.tile([128, 128], BF16, tag="band")
            nc.sync.dma_start(out=band, in_=band_d[:, :])
            ident = constp.tile([128, 128], BF16, tag="ident")
            make_identity(nc, ident)

            # PE warmup: ~32 throwaway transposes during the otherwise-dead
            # prologue window so the HAM un-throttles (K=8/8) before the
            # first real matmul burst.
            wrm = pwarmp.tile([128, 128], BF16, tag="warm")
            for _ in range(32):
                nc.tensor.transpose(wrm, ident, ident)

            # Preload act table set 6 (natural_log_exp_and_others): contains
            # Square, Sign, Ln, Exp, so the auto-placement pass never needs
            # another ACT_TABLE_LOAD (~2.7us each).
            nc.scalar.add_instruction(
                mybir.InstLoadActFuncSet(
                    name=nc.get_next_instruction_name(),
                    act_func_set_id=6,
                    ins=[],
                    outs=[],
                )
            )

            # ---- big persistent buffers ----
            alloc1 = bigp.tile([128, HP * WP], BF16, tag="alloc1")
            alloc2 = bigp.tile([128, HP * WP], BF16, tag="alloc2")
            a1v = alloc1.rearrange("p (hp wp) -> p hp wp", wp=WP)
            a2v = alloc2.rearrange("p (hp wp) -> p hp wp", wp=WP)
            lbc = bigp.tile([128, HW], BF16, tag="lbc")  # +ln(box), bcast over p
            s2p = bigp.tile([128, HP], BF16, tag="s2p")  # (w, padded h) sum x^2

            # zero borders of alloc1/alloc2: rows hp=0,129 and cols wp=0,129
            for av in (a1v, a2v):
                nc.vector.memset(av[:, 0, :], 0.0)
                nc.vector.memset(av[:, HP - 1, :], 0.0)
                nc.vector.memset(av[:, :, 0], 0.0)
                nc.vector.memset(av[:, :, WP - 1], 0.0)
            nc.vector.memset(s2p[:, 0:1], 0.0)
            nc.vector.memset(s2p[:, HP - 1 : HP], 0.0)

            def s2_slab(s):
                """sum-of-squares column of the norm image for slab s.

                Uses the hi half only: the missing 2*hi*lo term is ~2^-9
                relative on s2, ~a*5e-4 on the output."""
                h0 = s * SLAB
                packed = packed_tiles[s]
                xsq = slabp.tile([128, SLAB, C], BF16, tag="xsq")
                nc.gpsimd.tensor_tensor(
                    out=xsq,
                    in0=packed[:, :, 0, :],
                    in1=packed[:, :, 0, :],
                    op=mybir.AluOpType.mult,
                )
                with nc.allow_low_precision(reason="s2 bf16 ~5e-4 rel; out err ~a*2.5e-4"):
                    nc.vector.tensor_reduce(
                        out=s2p[:, 1 + h0 : 1 + h0 + SLAB],
                        in_=xsq,
                        axis=mybir.AxisListType.X,
                        op=mybir.AluOpType.add,
                    )

            def trans_slab(s):
                """transposes of slab s into the persistent padded images."""
                h0 = s * SLAB
                packed = packed_tiles.pop(s)
                for g in range(SLAB // 8):
                    ptr = ptrp.tile([128, 8, 128], BF16, tag="ptr")
                    for r in range(8):
                        hl = g * 8 + r
                        nc.tensor.transpose(
                            ptr[:, r, :],
                            packed[:, hl, :, :].rearrange("p t c -> p (t c)"),
                            ident,
                        )
                    hp0 = h0 + g * 8 + 1
                    nc.vector.tensor_copy(out=a1v[:, hp0 : hp0 + 8, 1 : 1 + W], in_=ptr)
                    nc.sync.dma_start(
                        out=a2v[0:64, hp0 : hp0 + 8, :],
                        in_=a1v[0:64, hp0 : hp0 + 8, :],
                    )
                    nc.sync.dma_start(
                        out=alloc2.rearrange("p (hp wp) -> p hp wp", wp=WP)[
                            64:128, hp0 : hp0 + 8, 0 : WP - 1
                        ],
                        in_=alloc1.rearrange("p (hp wp) -> p hp wp", wp=WP)[
                            0:64, hp0 : hp0 + 8, 1:WP
                        ],
                    )

            def box_batch(r0, nrows):
                """ln(box) for output rows [r0, r0+nrows) -> lbc columns.

                Needs s2p padded cols <= r0+nrows+1. Pixel range covered:
                [r0*W, (r0+nrows)*W).
                """
                timg = boxp_pool.tile([128, SLAB], BF16, tag="timg")
                t = timg[:, 0:nrows]
                # h-direction 3-sum (padded indices r0..r0+nrows+2)
                nc.vector.tensor_tensor(
                    out=t,
                    in0=s2p[:, r0 : r0 + nrows],
                    in1=s2p[:, r0 + 1 : r0 + 1 + nrows],
                    op=mybir.AluOpType.add,
                )
                nc.vector.tensor_tensor(
                    out=t,
                    in0=t,
                    in1=s2p[:, r0 + 2 : r0 + 2 + nrows],
                    op=mybir.AluOpType.add,
                )
                bx = pmiscp.tile([128, SLAB], F32, tag="bx")
                nc.tensor.matmul(bx[:, 0:nrows], band, t, start=True, stop=True)
                lpos = boxp_pool.tile([128, SLAB], BF16, tag="lpos")
                nc.scalar.activation(out=lpos[:, 0:nrows], in_=bx[:, 0:nrows], func=AF.Ln)
                lpt = pmiscp.tile([SLAB, 128], BF16, tag="lpt")
                nc.tensor.transpose(lpt[0:nrows, :], lpos[:, 0:nrows], ident)
                lrow = boxp_pool.tile([SLAB, 128], BF16, tag="lrow")
                nc.vector.tensor_copy(out=lrow[0:nrows, :], in_=lpt[0:nrows, :])
                flat = boxp_pool.tile([1, SLAB * W], BF16, tag="flat")
                nc.sync.dma_start(
                    out=flat.rearrange("o (h w) -> o h w", w=W)[:, 0:nrows, :],
                    in_=lrow[0:nrows, :],
                )
                nc.gpsimd.partition_broadcast(
                    lbc[:, r0 * W : (r0 + nrows) * W], flat[:, 0 : nrows * W]
                )

            # ---- GEMM + epilogue per output tile ----
            def emit_tile(j):
                hh = j * TILE_ROWS
                ps = pmmp.tile([128, TILE_ROWS, W], F32, tag="ps")
                first = True
                for ty in range(3):
                    for tx in range(3):
                        nc.tensor.matmul(
                            ps,
                            wt13[:, ty * 3 + tx, :],
                            a1v[:, hh + ty : hh + ty + TILE_ROWS, tx : tx + W],
                            start=first,
                            stop=False,
                        )
                        first = False
                for ty in range(3):
                    nc.tensor.matmul(
                        ps,
                        wt2p[:, ty, :],
                        a2v[:, hh + ty : hh + ty + TILE_ROWS, 0:W],
                        start=False,
                        stop=False,
                    )
                for ty in range(3):
                    nc.tensor.matmul(
                        ps,
                        ws_last[:, ty, :],
                        a2v[0:64, hh + ty : hh + ty + TILE_ROWS, 2 : 2 + W],
                        start=False,
                        stop=(ty == 2),
                    )
                psf = ps.rearrange("p r w -> p (r w)")
                # scalar: s^2 (set 6 is preloaded; no table reload)
                sq = epip.tile([128, NPIX], BF16, tag="sq")
                nc.scalar.activation(out=sq, in_=psf, func=AF.Square)
                # DVE: sign via bitwise ops (single PSUM input)
                sgn = epip.tile([128, NPIX], F32, tag="sgn")
                nc.vector.tensor_scalar(
                    out=sgn.bitcast(U32),
                    in0=psf.bitcast(U32),
                    scalar1=0x80000000,
                    scalar2=0x3F800000,
                    op0=mybir.AluOpType.bitwise_and,
                    op1=mybir.AluOpType.bitwise_or,
                )
                v = epip.tile([128, NPIX], BF16, tag="v")
                nc.scalar.activation(out=v, in_=sq, func=AF.Ln)
                v2 = epip.tile([128, NPIX], BF16, tag="v2")
                nc.vector.tensor_tensor(
                    out=v2,
                    in0=v,
                    in1=lbc[:, j * NPIX : (j + 1) * NPIX],
                    op=mybir.AluOpType.subtract,
                )
                t3 = epip.tile([128, NPIX], F32, tag="t3")
                nc.scalar.activation(out=t3, in_=v2, func=AF.Exp, scale=a2[:, :])
                o = epip.tile([128, NPIX], F32, tag="o")
                nc.gpsimd.tensor_tensor(
                    out=o, in0=t3, in1=sgn, op=mybir.AluOpType.mult
                )
                nc.sync.dma_start(out=out_d[:, j * NPIX : (j + 1) * NPIX], in_=o)

            # ---- fused streaming loop ----
            # s2/box/lbc run one slab AHEAD of the GEMM batch that consumes
            # them, emitted mid-GEMM, so the PE never head-blocks on
            # DVE/DMA at a slab boundary. The hi/lo split arrives pre-packed
            # from the host, so slabs go straight from DMA to transpose.
            s2_slab(0)
            box_batch(0, 12)  # lbc for tiles 0..2
            for s in range(NSLAB):
                issue_dma(s + 2)
                trans_slab(s)
                tiles = list(range(max(4 * s - 1, 0), 4 * s + 3))
                for j in tiles[:2]:
                    emit_tile(j)
                if s + 1 < NSLAB:
                    s2_slab(s + 1)
                    box_batch(16 * s + 12, 16)  # lbc for tiles 4s+3..4s+6
                else:
                    box_batch(124, 4)  # lbc for tile 31
                for j in tiles[2:]:
                    emit_tile(j)
            emit_tile(31)

    nc.compile()
    return nc


def _host_prep(w, p, q):
    EPS = 1e-12
    w64 = w[0].astype(np.float64)  # (576, 128)
    qv = (q.astype(np.float64) ** 2 / 10.0)[0]
    wn = np.sqrt(np.maximum((w64**2).sum(0), EPS)) + qv
    what = (w64 / wn).astype(np.float32)
    wh = what.astype(ml_dtypes.bfloat16)
    wl = (what - wh.astype(np.float32)).astype(ml_dtypes.bfloat16)

    def tap(a, k):
        return np.ascontiguousarray(a[k * 64 : (k + 1) * 64, :])

    wt13 = np.stack([np.vstack([tap(wh, k), tap(wh, k)]) for k in range(9)])
    wt2p = np.stack(
        [np.vstack([tap(wl, 3 * ty + 0), tap(wl, 3 * ty + 1)]) for ty in range(3)]
    )
    ws_last = np.stack([tap(wl, 2), tap(wl, 5), tap(wl, 8)])

    pe = np.exp(p.astype(np.float64) - p.astype(np.float64).max())
    a = pe / pe.sum()
    a2 = (a * 0.5).astype(np.float32).reshape(128, 1)

    band = np.zeros((128, 128), dtype=np.float32)
    for i in range(128):
        band[i, max(0, i - 1) : i + 2] = 1.0
    band = band.astype(ml_dtypes.bfloat16)
    return wt13, wt2p, ws_last, a2, band


LAST_RESULTS = None


def kernel(inputs, w, p, q):
    global LAST_RESULTS
    if "nc" not in _CACHE:
        _CACHE["nc"] = _build()
    nc = _CACHE["nc"]

    wt13, wt2p, ws_last, a2, band = _host_prep(w, p, q)
    # pre-split x into bf16 hi/lo pairs: [B, HW, 2, C] (same bytes as f32)
    xf = inputs.reshape(B, HW, C).astype(np.float32)
    xh = xf.astype(ml_dtypes.bfloat16)
    xl = (xf - xh.astype(np.float32)).astype(ml_dtypes.bfloat16)
    xs = np.ascontiguousarray(np.stack([xh, xl], axis=2))
    in_maps = [
        {
            "x": xs[b],
            "wt13": wt13,
            "wt2p": wt2p,
            "ws_last": ws_last,
            "a2": a2,
            "band": band,
        }
        for b in range(B)
    ]
    import os

    trace = bool(int(os.environ.get("KERNEL_TRACE", "0")))
    res = run_bass_kernel_spmd(nc, in_maps, core_ids=list(range(B)), trace=trace)
    LAST_RESULTS = res
    out = np.stack(
        [res.results[b]["out"].T.reshape(H, W, UNITS) for b in range(B)]
    ).astype(np.float32)
    return out



# revision 5
# speedup vs baseline: 2.4271x; 2.4271x over previous
"""Trainium2 Bass kernel for nn_CosSimConv2D (v2 - stall-free streaming).

Math (per sample b):
  s    = im2col3x3(x) @ w_hat           where w_hat = w / (||w||_col + qv)
  out  = sign(s) * exp(a_u/2 * (ln(s^2) - ln(box)))
  box  = 3x3 box-filter of per-pixel sum(x^2)  (= ||im2col row||^2)
  a    = softmax(p)

GEMM precision: x and w_hat are each split hi+lo in bf16; three product
terms (xh@wh + xh@wl + xl@wh) recover ~fp32-grade dot products.
Data-parallel over batch: core b computes sample b.

v2 changes vs v1 (which idled the PE 13-25us at every slab boundary and
kept the HAM clock-gate oscillating, so most matmuls ran at 1.2 GHz):
  - Host pre-builds the three padded 130x130 transposed images
    (img1=[xh;xl], img2=[xh;xh<<1col], img3=[xh<<2col;xh<<2col,+1row]),
    so slab input DMAs land straight in SBUF: no PE transposes, no
    SBUF->SBUF shift copies, no PSUM staging for im2col at all.
  - img3 packs the (ty=0,tx=2)+(ty=1,tx=2) lo-taps into one matmul:
    14 stationary weights per 8-row tile instead of 15.
  - Output tiles are 8 rows (N=1024, two N=512 matmuls per weight into
    a 2-bank PSUM tile), halving per-tile epilogue fixed costs.
  - The s2/box/ln(box) pipeline runs TWO slabs ahead of the GEMM that
    consumes it, so its long cross-engine latency chain never blocks
    the PE; GpSimd does only partition_broadcast (no library thrash).
  - Per-engine queues: sync=image DMAs+flat, scalar=xs2+output DMAs.
  - PE warmup uses real matmuls (transposes don't tick the HAM).
"""

import sys

sys.path.insert(0, "/opt/trn_rl_repo")

import numpy as np
import ml_dtypes

import concourse.bass as bass
import concourse.mybir as mybir
import concourse.tile as tile
from concourse import bacc
from concourse.bass_utils import run_bass_kernel_spmd
from concourse.masks import make_identity

BF16 = mybir.dt.bfloat16
F32 = mybir.dt.float32
U32 = mybir.dt.uint32
AF = mybir.ActivationFunctionType

B, H, W, C, UNITS = 8, 128, 128, 64, 128
HW = H * W  # 16384
HP, WP = H + 2, W + 2  # 130x130 padded image
SLAB = 16  # image rows per streaming slab
NSLAB = H // SLAB  # 8
PROWS = 8  # image rows per output pair-tile -> N = 1024
NPAIR = H // PROWS  # 16
NPIX = PROWS * W  # 1024
RING = 8  # lbc ring depth in pair-slots

_CACHE = {}


def _build():
    nc = bacc.Bacc("TRN2", target_bir_lowering=False, debug=False)

    img1_d = nc.dram_tensor("img1", [128, HP * WP], BF16, kind="ExternalInput")
    img2_d = nc.dram_tensor("img2", [128, HP * WP], BF16, kind="ExternalInput")
    img3_d = nc.dram_tensor("img3", [128, HP * WP], BF16, kind="ExternalInput")
    xs2_d = nc.dram_tensor("xs2", [128, H, C], BF16, kind="ExternalInput")
    wt13_d = nc.dram_tensor("wt13", [9, 128, UNITS], BF16, kind="ExternalInput")
    wt2p_d = nc.dram_tensor("wt2p", [3, 128, UNITS], BF16, kind="ExternalInput")
    wt3p_d = nc.dram_tensor("wt3p", [128, UNITS], BF16, kind="ExternalInput")
    wsl2_d = nc.dram_tensor("wsl2", [64, UNITS], BF16, kind="ExternalInput")
    a2_d = nc.dram_tensor("a2", [128, 1], F32, kind="ExternalInput")
    band_d = nc.dram_tensor("band", [128, 128], BF16, kind="ExternalInput")
    out_d = nc.dram_tensor("out", [128, HW], F32, kind="ExternalOutput")

    with tile.TileContext(nc) as tc:
        with (
            tc.tile_pool(name="const", bufs=1) as constp,
            tc.tile_pool(name="big", bufs=1) as bigp,
            tc.tile_pool(name="xs2p", bufs=4) as xs2p,
            tc.tile_pool(name="box", bufs=2) as boxp,
            tc.tile_pool(name="epi", bufs=2) as epip,
            tc.tile_pool(name="pmm", bufs=3, space="PSUM") as pmmp,
            tc.tile_pool(name="pmisc", bufs=2, space="PSUM") as pmiscp,
        ):
            # ---- constants (front of the queues; all small) ----
            wt13 = constp.tile([128, 9, UNITS], BF16, tag="wt13")
            nc.sync.dma_start(out=wt13, in_=wt13_d.ap().rearrange("t k u -> k t u"))
            wt2p = constp.tile([128, 3, UNITS], BF16, tag="wt2p")
            nc.scalar.dma_start(out=wt2p, in_=wt2p_d.ap().rearrange("t k u -> k t u"))
            wt3p = constp.tile([128, UNITS], BF16, tag="wt3p")
            nc.scalar.dma_start(out=wt3p, in_=wt3p_d[:, :])
            wsl2 = constp.tile([64, UNITS], BF16, tag="wsl2")
            nc.scalar.dma_start(out=wsl2, in_=wsl2_d[:, :])
            a2 = constp.tile([128, 1], F32, tag="a2")
            nc.scalar.dma_start(out=a2, in_=a2_d[:, :])
            band = constp.tile([128, 128], BF16, tag="band")
            nc.scalar.dma_start(out=band, in_=band_d[:, :])
            ident = constp.tile([128, 128], BF16, tag="ident")
            make_identity(nc, ident)

            # ---- big persistent buffers ----
            a1 = bigp.tile([128, HP * WP], BF16, tag="a1")
            a2i = bigp.tile([128, HP * WP], BF16, tag="a2i")
            a3 = bigp.tile([128, HP * WP], BF16, tag="a3")
            a1v = a1.rearrange("p (hp wp) -> p hp wp", wp=WP)
            a2v = a2i.rearrange("p (hp wp) -> p hp wp", wp=WP)
            a3v = a3.rearrange("p (hp wp) -> p hp wp", wp=WP)
            lbc = bigp.tile([128, RING * NPIX], BF16, tag="lbc")  # ln(box) ring
            s2p = bigp.tile([128, HP], BF16, tag="s2p")  # (w, padded h) sum x^2

            # ---- input slab DMAs ----
            # imgN rows for slab s: padded rows 1+16s .. 16+16s (borders
            # hp=0/129 are memset once below; wp borders are zero on host).
            img1v = img1_d.ap().rearrange("p (hp wp) -> p hp wp", wp=WP)
            img2v = img2_d.ap().rearrange("p (hp wp) -> p hp wp", wp=WP)
            img3v = img3_d.ap().rearrange("p (hp wp) -> p hp wp", wp=WP)
            xs2_tiles = {}

            def issue_dma(s):
                if s >= NSLAB:
                    return
                r0 = 1 + s * SLAB
                nc.sync.dma_start(out=a1v[:, r0 : r0 + SLAB, :], in_=img1v[:, r0 : r0 + SLAB, :])
                nc.sync.dma_start(out=a2v[:, r0 : r0 + SLAB, :], in_=img2v[:, r0 : r0 + SLAB, :])
                nc.sync.dma_start(out=a3v[:, r0 : r0 + SLAB, :], in_=img3v[:, r0 : r0 + SLAB, :])
                t = xs2p.tile([128, SLAB, C], BF16, tag="xs2")
                nc.scalar.dma_start(out=t, in_=xs2_d[:, s * SLAB : (s + 1) * SLAB, :])
                xs2_tiles[s] = t

            issue_dma(0)
            issue_dma(1)
            issue_dma(2)

            # PE warmup: real matmuls tick the HAM activity monitor
            # (transposes do not), so the array is at K=8/8 by the time
            # the first GEMM's data lands.
            wrm = pmiscp.tile([128, 128], F32, tag="misc")
            for _ in range(24):
                nc.tensor.matmul(wrm, ident, ident, start=True, stop=True)

            # Preload act table set 6 (natural_log_exp_and_others):
            # Square, Sign, Ln, Exp -> no ACT_TABLE_LOAD mid-kernel.
            nc.scalar.add_instruction(
                mybir.InstLoadActFuncSet(
                    name=nc.get_next_instruction_name(),
                    act_func_set_id=6,
                    ins=[],
                    outs=[],
                )
            )

            # zero the hp borders the DMAs never write
            for av in (a1v, a2v, a3v):
                nc.vector.memset(av[:, 0, :], 0.0)
                nc.vector.memset(av[:, HP - 1, :], 0.0)
            nc.vector.memset(s2p[:, 0:1], 0.0)
            nc.vector.memset(s2p[:, HP - 1 : HP], 0.0)

            def s2_slab(s):
                """sum-of-squares column of the norm image for slab s (hi only:
                the missing 2*hi*lo term is ~2^-9 relative on s2, ~a*5e-4 out)."""
                t = xs2_tiles.pop(s)
                xsq = xs2p.tile([128, SLAB, C], BF16, tag="xsq")
                nc.scalar.activation(out=xsq, in_=t, func=AF.Square)
                with nc.allow_low_precision(reason="s2 bf16 ~5e-4 rel; out err ~a*2.5e-4"):
                    nc.vector.tensor_reduce(
                        out=s2p[:, 1 + s * SLAB : 1 + (s + 1) * SLAB],
                        in_=xsq,
                        axis=mybir.AxisListType.X,
                        op=mybir.AluOpType.add,
                    )

            def box_pair(p):
                """ln(box) for output pair p (rows 8p..8p+7) -> lbc ring slot."""
                r0 = 8 * p
                timg = boxp.tile([128, PROWS], BF16, tag="timg")
                nc.vector.tensor_tensor(
                    out=timg,
                    in0=s2p[:, r0 : r0 + PROWS],
                    in1=s2p[:, r0 + 1 : r0 + 1 + PROWS],
                    op=mybir.AluOpType.add,
                )
                nc.vector.tensor_tensor(
                    out=timg,
                    in0=timg,
                    in1=s2p[:, r0 + 2 : r0 + 2 + PROWS],
                    op=mybir.AluOpType.add,
                )
                bx = pmiscp.tile([128, PROWS], F32, tag="misc")
                nc.tensor.matmul(bx, band, timg, start=True, stop=True)
                lpos = boxp.tile([128, PROWS], BF16, tag="lpos")
                nc.scalar.activation(out=lpos, in_=bx, func=AF.Ln)
                lpt = pmiscp.tile([PROWS, 128], BF16, tag="misc")
                nc.tensor.transpose(lpt, lpos, ident)
                lrow = boxp.tile([PROWS, 128], BF16, tag="lrow")
                nc.vector.tensor_copy(out=lrow, in_=lpt)
                flat = boxp.tile([1, NPIX], BF16, tag="flat")
                nc.sync.dma_start(
                    out=flat.rearrange("o (h w) -> o h w", w=W),
                    in_=lrow,
                )
                slot = p % RING
                nc.gpsimd.partition_broadcast(
                    lbc[:, slot * NPIX : (slot + 1) * NPIX], flat[:, 0:NPIX]
                )

            # ---- GEMM + epilogue per 8-row output pair-tile ----
            def emit_pair(p):
                hh = 8 * p
                ps = pmmp.tile([128, 2, 512], F32, tag="ps")
                # 14 stationary weights; each does two N=512 matmuls
                # (rows hh..hh+3 and hh+4..hh+7) so consecutive matmuls
                # ping-pong PSUM banks and weights load once per pair.
                movs = []
                for ty in range(3):
                    for tx in range(3):
                        movs.append(
                            (wt13[:, 3 * ty + tx, :],
                             lambda k, ty=ty, tx=tx: a1v[:, hh + ty + 4 * k : hh + ty + 4 * k + 4, tx : tx + W])
                        )
                for ty in range(3):
                    movs.append(
                        (wt2p[:, ty, :],
                         lambda k, ty=ty: a2v[:, hh + ty + 4 * k : hh + ty + 4 * k + 4, 0:W])
                    )
                movs.append((wt3p, lambda k: a3v[:, hh + 4 * k : hh + 4 * k + 4, 0:W]))
                movs.append((wsl2, lambda k: a3v[0:64, hh + 2 + 4 * k : hh + 2 + 4 * k + 4, 0:W]))
                nmov = len(movs)
                for wi, (wap, mv) in enumerate(movs):
                    for k in range(2):
                        nc.tensor.matmul(
                            ps[:, k, :],
                            wap,
                            mv(k),
                            start=(wi == 0),
                            stop=(wi == nmov - 1),
                        )
                psf = ps.rearrange("p a n -> p (a n)")
                # scalar: s^2 (set 6 preloaded)
                sq = epip.tile([128, NPIX], BF16, tag="sq")
                nc.scalar.activation(out=sq, in_=psf, func=AF.Square)
                # DVE: sign via bitwise ops (single PSUM input)
                sgn = epip.tile([128, NPIX], F32, tag="sgn")
                nc.vector.tensor_scalar(
                    out=sgn.bitcast(U32),
                    in0=psf.bitcast(U32),
                    scalar1=0x80000000,
                    scalar2=0x3F800000,
                    op0=mybir.AluOpType.bitwise_and,
                    op1=mybir.AluOpType.bitwise_or,
                )
                v = epip.tile([128, NPIX], BF16, tag="v")
                nc.scalar.activation(out=v, in_=sq, func=AF.Ln)
                slot = p % RING
                v2 = epip.tile([128, NPIX], BF16, tag="v2")
                nc.vector.tensor_tensor(
                    out=v2,
                    in0=v,
                    in1=lbc[:, slot * NPIX : (slot + 1) * NPIX],
                    op=mybir.AluOpType.subtract,
                )
                t3 = epip.tile([128, NPIX], F32, tag="t3")
                nc.scalar.activation(out=t3, in_=v2, func=AF.Exp, scale=a2[:, :])
                o = epip.tile([128, NPIX], F32, tag="o")
                nc.vector.tensor_tensor(out=o, in0=t3, in1=sgn, op=mybir.AluOpType.mult)
                nc.scalar.dma_start(out=out_d[:, p * NPIX : (p + 1) * NPIX], in_=o)

            # ---- prologue: s2/box two slabs ahead of the GEMM ----
            s2_slab(0)
            s2_slab(1)
            for p in (0, 1, 2):
                box_pair(p)

            # ---- fused streaming loop ----
            for s in range(NSLAB):
                issue_dma(s + 3)
                if s + 2 < NSLAB:
                    s2_slab(s + 2)
                for p in (2 * s + 3, 2 * s + 4):
                    if p < NPAIR:
                        box_pair(p)
                if s > 0:
                    emit_pair(2 * s - 1)
                emit_pair(2 * s)
            emit_pair(NPAIR - 1)

    nc.compile()
    return nc


def _host_prep(w, p, q):
    EPS = 1e-12
    w64 = w[0].astype(np.float64)  # (576, 128)
    qv = (q.astype(np.float64) ** 2 / 10.0)[0]
    wn = np.sqrt(np.maximum((w64**2).sum(0), EPS)) + qv
    what = (w64 / wn).astype(np.float32)
    wh = what.astype(ml_dtypes.bfloat16)
    wl = (what - wh.astype(np.float32)).astype(ml_dtypes.bfloat16)

    def tap(a, k):
        return np.ascontiguousarray(a[k * 64 : (k + 1) * 64, :])

    wt13 = np.stack([np.vstack([tap(wh, k), tap(wh, k)]) for k in range(9)])
    wt2p = np.stack(
        [np.vstack([tap(wl, 3 * ty + 0), tap(wl, 3 * ty + 1)]) for ty in range(3)]
    )
    wt3p = np.vstack([tap(wl, 2), tap(wl, 5)])
    wsl2 = tap(wl, 8)

    pe = np.exp(p.astype(np.float64) - p.astype(np.float64).max())
    a = pe / pe.sum()
    a2 = (a * 0.5).astype(np.float32).reshape(128, 1)

    band = np.zeros((128, 128), dtype=np.float32)
    for i in range(128):
        band[i, max(0, i - 1) : i + 2] = 1.0
    band = band.astype(ml_dtypes.bfloat16)
    return wt13, wt2p, wt3p, wsl2, a2, band


def _host_images(xf):
    """xf: (H, W, C) fp32 one sample -> img1, img2, img3 (128, HP*WP) bf16, xs2."""
    xh = xf.astype(ml_dtypes.bfloat16)
    xl = (xf - xh.astype(np.float32)).astype(ml_dtypes.bfloat16)
    ph = np.zeros((HP, WP + 2, C), dtype=ml_dtypes.bfloat16)  # extra 2 cols for shifts
    pl = np.zeros((HP, WP, C), dtype=ml_dtypes.bfloat16)
    ph[1 : H + 1, 1 : W + 1] = xh
    pl[1 : H + 1, 1 : W + 1] = xl

    def T(img):  # (HP, WP, C) -> (C, HP*WP)
        return np.ascontiguousarray(img.transpose(2, 0, 1)).reshape(C, HP * WP)

    img1 = np.concatenate([T(ph[:, :WP]), T(pl)], axis=0)
    img2 = np.concatenate([T(ph[:, :WP]), T(ph[:, 1 : WP + 1])], axis=0)
    # img3: upper = xh shifted +2 cols; lower = same, also shifted +1 row
    up = ph[:, 2 : WP + 2]
    low = np.zeros_like(up)
    low[: HP - 1] = up[1:HP]
    img3 = np.concatenate([T(up), T(low)], axis=0)
    xs2 = np.ascontiguousarray(xh.transpose(1, 0, 2))  # (W, H, C)
    return img1, img2, img3, xs2


LAST_RESULTS = None


def kernel(inputs, w, p, q):
    global LAST_RESULTS
    if "nc" not in _CACHE:
        _CACHE["nc"] = _build()
    nc = _CACHE["nc"]

    wt13, wt2p, wt3p, wsl2, a2, band = _host_prep(w, p, q)
    xf = np.asarray(inputs, dtype=np.float32)
    in_maps = []
    for b in range(B):
        img1, img2, img3, xs2 = _host_images(xf[b])
        in_maps.append(
            {
                "img1": img1,
                "img2": img2,
                "img3": img3,
                "xs2": xs2,
                "wt13": wt13,
                "wt2p": wt2p,
                "wt3p": wt3p,
                "wsl2": wsl2,
                "a2": a2,
                "band": band,
            }
        )
    import os

    trace = bool(int(os.environ.get("KERNEL_TRACE", "0")))
    res = run_bass_kernel_spmd(nc, in_maps, core_ids=list(range(B)), trace=trace)
    LAST_RESULTS = res
    out = np.stack(
        [res.results[b]["out"].T.reshape(H, W, UNITS) for b in range(B)]
    ).astype(np.float32)
    return out
